# revision 1
# baseline (speedup 1.0000x reference)
"""CfC (closed-form continuous-time) 3-layer NCP encoder on 8 Trainium2 cores.

Strategy: data-parallel over batch (256 -> 32 per core), weights replicated.
Per core the T=1024 recurrence runs fully on-chip:
  - state kept feature-major in SBUF via PE transposes each step
  - gate matmuls: x_cat stationary [K,32], fused [ff1|ff2|tab] weights moving
  - all three layers' pre-activations land in one contiguous PSUM tile so
    tanh/sigmoid are 2 ACT instructions per step
  - blend split across DVE/GPSIMD; x_t streamed from HBM per step (2-step
    prefetch into the batch-major H tile, transposed for free by T1)
"""
import sys, os

sys.path.insert(0, "/opt/trn_rl_repo")
os.environ.setdefault("JAX_PLATFORMS", "")
os.environ.setdefault("MYCRO_LOCAL_CACHE", "1")

import numpy as np
import ml_dtypes

N_CORES = 8
B, T_FULL, D_IN = 256, 1024, 64
BL = B // N_CORES  # 32
H0, H1, H2 = 135, 89, 32

_NC_CACHE = {}


def _build(T, debug=False):
    ABL = os.environ.get("ABLATE", "none")
    REP = int(os.environ.get("REPEAT", "1"))
    import concourse.tile as tile
    from concourse import bacc, mybir

    F32 = mybir.dt.float32
    BF16 = mybir.dt.bfloat16
    AF = mybir.ActivationFunctionType

    nc = bacc.Bacc("TRN2", target_bir_lowering=False, debug=False,
                   enable_asserts=True, num_devices=N_CORES)

    d_x = nc.dram_tensor("x_bm", [BL, T * D_IN], F32, kind="ExternalInput").ap()
    d_xci = nc.dram_tensor("xc_init", [128, 96], BF16, kind="ExternalInput").ap()
    d_w0a = nc.dram_tensor("w0a", [128, 3 * H0], BF16, kind="ExternalInput").ap()
    d_w0b = nc.dram_tensor("w0b", [72, 3 * H0], BF16, kind="ExternalInput").ap()
    d_w1c1 = nc.dram_tensor("w1c1", [128, 3 * H1], BF16, kind="ExternalInput").ap()
    d_w1c2 = nc.dram_tensor("w1c2", [72, 3 * H1], BF16, kind="ExternalInput").ap()
    d_w1c3 = nc.dram_tensor("w1c3", [90, 3 * H1], BF16, kind="ExternalInput").ap()
    d_w2 = nc.dram_tensor("w2", [128, 3 * H2], BF16, kind="ExternalInput").ap()
    d_fc = [nc.dram_tensor(f"fc{p}{m}", [128 if p != "B" else 72, 128], F32,
                           kind="ExternalInput").ap()
            for p in ("A", "B", "C") for m in (0, 1)]
    d_id = nc.dram_tensor("ident", [32, 32], F32, kind="ExternalInput").ap()
    d_out = nc.dram_tensor("out", [128, 64], F32, kind="ExternalOutput").ap()
    if debug:
        d_dxc = nc.dram_tensor("dbg_xc", [128, 96], F32, kind="ExternalOutput").ap()
        d_dh = nc.dram_tensor("dbg_h", [32, 352], F32, kind="ExternalOutput").ap()
        d_dff = nc.dram_tensor("dbg_ff", [32, 512], F32, kind="ExternalOutput").ap()
        d_dsg = nc.dram_tensor("dbg_sg", [32, 256], F32, kind="ExternalOutput").ap()
        d_dpg = nc.dram_tensor("dbg_pg", [32, 768], F32, kind="ExternalOutput").ap()

    with tile.TileContext(nc, trace_sim=False) as tc:
        with tc.tile_pool(name="persist", bufs=1) as pp, \
             tc.tile_pool(name="psum", bufs=1, space="PSUM") as psp:
            # --- persistent SBUF ---
            sW0a = pp.tile([128, 3 * H0], BF16)
            sW0b = pp.tile([72, 3 * H0], BF16)
            sW1c1 = pp.tile([128, 3 * H1], BF16)
            sW1c2 = pp.tile([72, 3 * H1], BF16)
            sW1c3 = pp.tile([90, 3 * H1], BF16)
            sW2 = pp.tile([128, 3 * H2], BF16)
            sFc = [pp.tile([128 if p != "B" else 72, 128], F32, name=f"fc{p}{m}")
                   for p in ("A", "B", "C") for m in (0, 1)]
            sId = pp.tile([32, 32], F32)
            sH = [pp.tile([32, 352], F32, name=f"H{i}") for i in range(2)]
            sFF = [pp.tile([32, 512], F32, name=f"FF{i}") for i in range(2)]
            sSG = [pp.tile([32, 256], F32, name=f"SG{i}") for i in range(2)]
            sD = [pp.tile([32, 256], F32, name=f"D{i}") for i in range(2)]
            sM = [pp.tile([32, 256], F32, name=f"M{i}") for i in range(2)]
            sXC = [pp.tile([128, 96], BF16, name=f"XC{i}") for i in range(2)]
            sXCf = [pp.tile([128, 96], F32, name=f"XCf{i}") for i in range(2)]
            sOut = pp.tile([128, 64], F32)
            sCpg = pp.tile([32, 768], F32)
            sCh = pp.tile([32, 352], F32)
            sCxc = pp.tile([128, 96], BF16)
            # --- PSUM ---
            pGate = [psp.tile([32, 768], F32, name=f"PG{i}") for i in range(2)]
            pT = [psp.tile([128, 96], F32, name=f"PT{i}") for i in range(2)]
            pFC = psp.tile([128, 64], F32)

            # --- load weights ---
            for dst, src in [(sW0a, d_w0a), (sW0b, d_w0b), (sW1c1, d_w1c1),
                             (sW1c2, d_w1c2), (sW1c3, d_w1c3), (sW2, d_w2),
                             (sId, d_id)] + list(zip(sFc, d_fc)):
                nc.sync.dma_start(dst[:], src)

            # --- init state ---
            for tl in (sCpg, sCh, sCxc):
                nc.vector.memset(tl[:], 0.01)
            for tl in sFF + sSG + sD + sM:
                nc.vector.memset(tl[:], 0.0)
            for h in sH:
                nc.vector.memset(h[:], 0.0)
                nc.vector.memset(h[:, 199:200], 1.0)
                nc.vector.memset(h[:, 313:314], 1.0)
            nc.sync.dma_start(sXC[1][:], d_xci)
            if T > 1:
                nc.sync.dma_start(sH[0][:, 0:64], d_x[:, D_IN:2 * D_IN])

            mm = nc.tensor.matmul

            # Layer wavefront: iteration i computes L0@step i, L1@step i-1,
            # L2@step i-2 so each layer reads the previous iteration's
            # transposed state (which holds exactly what it needs).
            import contextlib
            rep_ctx = tc.For_i(0, REP, 1) if REP > 1 else contextlib.nullcontext()
            with rep_ctx:
              for i in range(T + 2):
                  p = i % 2
                  xc = sXC[1 - p] if ABL != "nomm" else sCxc
                  pg, pt, h, ff, sg = pGate[p], pT[p], sH[p], sFF[p], sSG[p]

                  # x prefetch two iterations ahead into the *other* H tile
                  if i + 2 < T and ABL != "nodma":
                      nc.sync.dma_start(sH[1 - p][:, 0:64],
                                        d_x[:, (i + 2) * D_IN:(i + 3) * D_IN])

                  l0_on, l1_on, l2_on = i < T, 1 <= i <= T, 2 <= i
                  if ABL == "nomm":
                      l0_on = l1_on = l2_on = False
                  # --- gate matmuls (ff gates first so tanh starts early) ---
                  def mm_l0(g, c0, c1):
                      mm(pg[:, c0:c1], xc[0:128, 0:32],
                         sW0a[:, g * H0:(g + 1) * H0], start=True, stop=False)
                      mm(pg[:, c0:c1], xc[0:72, 32:64],
                         sW0b[:, g * H0:(g + 1) * H0], start=False, stop=True)
                  def mm_l1(g, c0, c1):
                      mm(pg[:, c0:c1], xc[64:128, 0:32],
                         sW1c1[64:128, g * H1:(g + 1) * H1], start=True, stop=False)
                      mm(pg[:, c0:c1], xc[0:72, 32:64],
                         sW1c2[0:72, g * H1:(g + 1) * H1], start=False, stop=False)
                      mm(pg[:, c0:c1], xc[0:90, 64:96],
                         sW1c3[0:90, g * H1:(g + 1) * H1], start=False, stop=True)
                  def mm_l2(g, c0, c1):
                      mm(pg[:, c0:c1], xc[0:128, 64:96],
                         sW2[:, g * H2:(g + 1) * H2], start=True, stop=True)
                  if l0_on:
                      mm_l0(0, 0, 135); mm_l0(1, 256, 391)
                  if l1_on:
                      mm_l1(0, 135, 224); mm_l1(1, 391, 480)
                  if l2_on:
                      mm_l2(0, 224, 256); mm_l2(1, 480, 512)
                  if l0_on:
                      mm_l0(2, 512, 647)
                  if l1_on:
                      mm_l1(2, 647, 736)
                  if l2_on:
                      mm_l2(2, 736, 768)

                  # --- activations (2 fused ops; stale regions unused) ---
                  pgsrc = pg if ABL != "noact" else sCpg
                  _sgf = AF.Tanh if os.environ.get("SGFN") == "tanh" else AF.Sigmoid
                  nc.scalar.activation(ff[:], pgsrc[:, 0:512], AF.Tanh)
                  nc.scalar.activation(sg[:], pgsrc[:, 512:768], _sgf)

                  # --- blend: h = ff1 + sg*(ff2-ff1) ---
                  d_, m_ = sD[p], sM[p]
                  if ABL == "noblend":
                      l0_on = l1_on = l2_on = False
                  if ABL != "noblend":
                      nc.vector.tensor_sub(d_[:], ff[:, 256:512], ff[:, 0:256])
                      nc.vector.tensor_mul(m_[:], sg[:], d_[:])
                  if l0_on:
                      nc.vector.tensor_add(h[:, 64:199], ff[:, 0:135], m_[:, 0:135])
                  if l1_on:
                      nc.gpsimd.tensor_add(h[:, 224:313], ff[:, 135:224],
                                           m_[:, 135:224])
                  if l2_on:
                      nc.gpsimd.tensor_add(h[:, 320:352], ff[:, 224:256],
                                           m_[:, 224:256])

                  # --- transpose state to feature-major ---
                  hsrc = h if ABL != "notr" else sCh
                  nc.tensor.transpose(pt[0:128, 0:32], hsrc[:, 0:128], sId[:])
                  nc.tensor.transpose(pt[0:72, 32:64], hsrc[:, 128:200], sId[:])
                  nc.tensor.transpose(pt[0:128, 64:96], hsrc[:, 224:352], sId[:])
                  nc.vector.tensor_copy(sXC[p][:], pt[:])
                  if i >= T - 1:
                      nc.vector.tensor_copy(sXCf[p][:], pt[:])

            # --- final FC: out.T[u,b] = fc_W @ hn + fc_b ---
            # each layer's final state lives in a different xcat parity
            xc0f = sXCf[(T + 1) % 2]
            xc1f = sXCf[T % 2]
            xc2f = sXCf[(T + 1) % 2]
            for mchunk in range(2):
                o = pFC[:, mchunk * 32:(mchunk + 1) * 32]
                mm(o, sFc[0 + mchunk][64:128, :], xc0f[64:128, 0:32],
                   start=True, stop=False)
                mm(o, sFc[2 + mchunk][0:72, :], xc0f[0:72, 32:64],
                   start=False, stop=False)
                mm(o, sFc[4 + mchunk][0:90, :], xc1f[0:90, 64:96],
                   start=False, stop=False)
                mm(o, sFc[4 + mchunk][96:128, :], xc2f[96:128, 64:96],
                   start=False, stop=True, tile_position=(96, 0))
            if debug:
                lp = (T + 1) % 2
                nc.sync.dma_start(d_dxc, sXC[lp][:])
                nc.sync.dma_start(d_dh, sH[lp][:])
                nc.sync.dma_start(d_dff, sFF[lp][:])
                nc.sync.dma_start(d_dsg, sSG[lp][:])
                sDbgPg = pp.tile([32, 768], F32, name="sDbgPg")
                nc.scalar.activation(sDbgPg[:], pGate[lp][:], AF.Copy)
                nc.sync.dma_start(d_dpg, sDbgPg[:])
            nc.vector.tensor_copy(sOut[:], pFC[:])
            nc.sync.dma_start(d_out, sOut[:])

    nc.compile()
    return nc


def _prep_weights(inputs):
    """Host-side weight packing (numpy). Returns dict of shared tiles."""
    import ml_dtypes
    bf = ml_dtypes.bfloat16
    f = np.float32
    out = {}
    # layer 0: ref cols [x(0:64) | h0(64:199)]
    ff1 = (inputs["W1_0"] * inputs["mask0"]).astype(f)   # [135, 199]
    ff2 = (inputs["W2_0"] * inputs["mask0"]).astype(f)
    tab = (inputs["Wa_0"] + inputs["Wb_0"]).astype(f)
    b = [inputs["b1_0"].astype(f), inputs["b2_0"].astype(f),
         (inputs["ba_0"] + inputs["bb_0"]).astype(f)]
    w0a = np.concatenate([g[:, 0:128].T for g in (ff1, ff2, tab)], axis=1)
    w0b = np.zeros((72, 3 * H0), f)
    w0b[0:71] = np.concatenate([g[:, 128:199].T for g in (ff1, ff2, tab)], axis=1)
    w0b[71] = np.concatenate(b)
    out["w0a"], out["w0b"] = np.ascontiguousarray(w0a).astype(bf), w0b.astype(bf)

    # layer 1: ref cols [h0(0:135) | h1(135:224)]
    ff1 = (inputs["W1_1"] * inputs["mask1"]).astype(f)
    ff2 = (inputs["W2_1"] * inputs["mask1"]).astype(f)
    tab = (inputs["Wa_1"] + inputs["Wb_1"]).astype(f)
    b = [inputs["b1_1"].astype(f), inputs["b2_1"].astype(f),
         (inputs["ba_1"] + inputs["bb_1"]).astype(f)]
    w1c1 = np.zeros((128, 3 * H1), f)
    w1c1[64:128] = np.concatenate([g[:, 0:64].T for g in (ff1, ff2, tab)], axis=1)
    w1c2 = np.zeros((72, 3 * H1), f)
    w1c2[0:71] = np.concatenate([g[:, 64:135].T for g in (ff1, ff2, tab)], axis=1)
    w1c3 = np.zeros((90, 3 * H1), f)
    w1c3[0:89] = np.concatenate([g[:, 135:224].T for g in (ff1, ff2, tab)], axis=1)
    w1c3[89] = np.concatenate(b)
    out["w1c1"], out["w1c2"], out["w1c3"] = w1c1.astype(bf), w1c2.astype(bf), w1c3.astype(bf)

    # layer 2: ref cols [h1(0:89) | h2(89:121)]
    ff1 = (inputs["W1_2"] * inputs["mask2"]).astype(f)
    ff2 = (inputs["W2_2"] * inputs["mask2"]).astype(f)
    tab = (inputs["Wa_2"] + inputs["Wb_2"]).astype(f)
    b = [inputs["b1_2"].astype(f), inputs["b2_2"].astype(f),
         (inputs["ba_2"] + inputs["bb_2"]).astype(f)]
    w2 = np.zeros((128, 3 * H2), f)
    w2[0:89] = np.concatenate([g[:, 0:89].T for g in (ff1, ff2, tab)], axis=1)
    w2[89] = np.concatenate(b)
    w2[96:128] = np.concatenate([g[:, 89:121].T for g in (ff1, ff2, tab)], axis=1)
    out["w2"] = w2.astype(bf)

    # fc pieces; hn ref order = [h0 | h1 | h2]
    fcW, fcb = inputs["fc_W"].astype(f), inputs["fc_b"].astype(f)
    for m in range(2):
        sl = slice(128 * m, 128 * (m + 1))
        a = np.zeros((128, 128), f)
        a[64:128] = fcW[sl, 0:64].T
        bb_ = np.zeros((72, 128), f)
        bb_[0:71] = fcW[sl, 64:135].T
        bb_[71] = fcb[sl]
        c = np.zeros((128, 128), f)
        c[0:89] = fcW[sl, 135:224].T
        c[96:128] = fcW[sl, 224:256].T
        out[f"fcA{m}"], out[f"fcB{m}"], out[f"fcC{m}"] = a, bb_, c

    out["ident"] = np.eye(32, dtype=f)
    return out


def kernel(debug=False, **inputs):
    from concourse.bass_utils import run_bass_kernel_spmd

    T = inputs["x"].shape[1]
    key = (T, debug, os.environ.get("ABLATE", "none"), os.environ.get("REPEAT", "1"), os.environ.get("SGFN", ""))
    if key not in _NC_CACHE:
        _NC_CACHE[key] = _build(T, debug)
    nc = _NC_CACHE[key]

    shared = _prep_weights(inputs)
    x = np.asarray(inputs["x"], dtype=np.float32)
    in_maps = []
    for c in range(N_CORES):
        xc = np.ascontiguousarray(x[c * BL:(c + 1) * BL])  # [32, T, 64]
        m = dict(shared)
        m["x_bm"] = xc.reshape(BL, T * D_IN)
        xci = np.zeros((128, 96), np.float32)
        xci[0:64, 0:32] = xc[:, 0, :].T
        xci[71, 32:64] = 1.0
        xci[89, 64:96] = 1.0
        m["xc_init"] = xci.astype(ml_dtypes.bfloat16)
        in_maps.append(m)

    res = run_bass_kernel_spmd(nc, in_maps, list(range(N_CORES)))

    if debug:
        return res
    out = np.zeros((B, 256), np.float32)
    for c in range(N_CORES):
        o = res.results[c]["out"]  # [128, 64]
        blk = np.concatenate([o[:, 0:32], o[:, 32:64]], axis=0)  # [256, 32]
        out[c * BL:(c + 1) * BL] = blk.T
    return out

